# revision 1
# baseline (speedup 1.0000x reference)
"""Trainium2 Bass kernel for nn_AutoregressiveCDF (MADE + rational-quadratic
spline CDF, product over features).

Strategy: pure data-parallel over 8 NeuronCores (batch 16384 -> 8 x 2048),
weights replicated.  Per core:
  Phase A: transpose predicates/contexts via PE, run the MADE trunk as
           hidden-on-partition GEMMs (fp32r on the PE), activations on ACT.
  Phase B: output GEMM t @ W_out in [batch-part, feature-free] orientation,
           PSUM consumed directly by ACT exp/softplus; spline evaluated with a
           monotone-mask formulation (no gather): for each feature the bin
           index idx satisfies u_j = 1[x >= edge_j], and any per-bin quantity
           at idx is a masked sum  sum_j u_j * T_j  (segmented reduce on DVE).
"""

import numpy as np
from contextlib import ExitStack

import concourse.bass as bass
import concourse.bacc as bacc
import concourse.tile as tile
from concourse import mybir
from concourse.bass_utils import run_bass_kernel_spmd

F32 = mybir.dt.float32

# problem sizes (hardcoded per contract)
B, F, H, C = 16384, 64, 512, 512
NB = 30
MULT = 3 * NB + 1            # 91
NBLOCKS = 3
NCORES = 8
MIN_BIN = 1e-3
MIN_DERIV = 1e-3
CFREE = float(1.0 - MIN_BIN * NB)         # softmax mass after min-bin affine
SCALE = float(np.float32(1.0 / np.sqrt(H)))
FH = F // 2                  # features per half (32)
WOH = FH * MULT              # 2912 W_out cols per half
KH = H // 128                # 4 hidden chunks

# knobs (test.py may override module globals before calling kernel())
MM_DT = mybir.dt.float32r    # PE dtype: float32r (fast) or float32 (safe)
USE_SCANMUL = True           # custom DVE scan-mul gathers vs stock mult+reduce
TRACE = False
LAST_RESULTS = None          # BassKernelResults of the most recent run

_CACHE = {}


def _masks():
    d_in = np.arange(1, F + 1)
    d_h = np.arange(H) % max(1, F - 1) + min(1, F - 1)
    m_in = (d_h[None, :] >= d_in[:, None]).astype(np.float32)
    m_hh = (d_h[None, :] >= d_h[:, None]).astype(np.float32)
    d_out = np.repeat(d_in, MULT)
    m_out = (d_out[None, :] > d_h[:, None]).astype(np.float32)
    return m_in, m_hh, m_out


def _scan_mul_ref(in0, in1, s0, s1, imm2):
    a = np.asarray(in0, np.float32).reshape(np.asarray(in0).shape[0], -1)
    b = np.asarray(in1, np.float32).reshape(a.shape)
    return np.cumsum(a * b, axis=1, dtype=np.float32).reshape(
        np.asarray(in0).shape)


def _cumsum_ref(in0, in1, s0, s1, imm2):
    a = np.asarray(in0, np.float32).reshape(np.asarray(in0).shape[0], -1)
    return np.cumsum(a, axis=1, dtype=np.float32).reshape(
        np.asarray(in0).shape)


def _register_scan_mul():
    """Register a fused multiply+prefix-sum DVE op: out = cumsum(in0*in1).

    One DVE pass per masked-sum gather instead of tensor_tensor +
    tensor_reduce; per-feature sums are recovered from segment-boundary
    differences of the chained running sum."""
    import concourse.dve_ops as dve_ops
    from concourse.dve_spec import Spec, Src0, Src1, scan, AluOp, lower
    from concourse.dve_uop import DveOpSpec
    for op in dve_ops.OPS:
        if op.name == "SCAN_MUL_ANT":
            return op
    spec = Spec(body=scan(AluOp.ADD, Src0 * Src1), reference=_scan_mul_ref)
    row = max(dve_ops._SUB_OPCODE_FOR_NAME.values()) + 1
    assert row < 0x20
    shas = {}
    for ver in ("v3", "v4"):
        u = lower(spec, ver=ver)
        shas[ver] = DveOpSpec(name="SCAN_MUL_ANT", opcode=row, uops=u,
                              rd1_en=True).sha(ver)
    op = dve_ops.DveOp("SCAN_MUL_ANT", spec, subdim=False, uops_sha=shas)
    dve_ops.OPS.append(op)
    dve_ops.CUSTOM_DVE_SPECS["SCAN_MUL_ANT"] = spec
    dve_ops._SUB_OPCODE_FOR_NAME["SCAN_MUL_ANT"] = row

    spec2 = Spec(body=scan(AluOp.ADD, Src0), reference=_cumsum_ref)
    row2 = row + 1
    assert row2 < 0x20
    shas2 = {}
    for ver in ("v3", "v4"):
        u2 = lower(spec2, ver=ver)
        shas2[ver] = DveOpSpec(name="CUMSUM_ANT", opcode=row2, uops=u2,
                               rd1_en=False).sha(ver)
    op2 = dve_ops.DveOp("CUMSUM_ANT", spec2, subdim=False, uops_sha=shas2)
    dve_ops.OPS.append(op2)
    dve_ops.CUSTOM_DVE_SPECS["CUMSUM_ANT"] = spec2
    dve_ops._SUB_OPCODE_FOR_NAME["CUMSUM_ANT"] = row2
    return op, op2


class _Bacc(bacc.Bacc):
    """Bacc with a trimmed activation-table list so Exp and Ln share one
    table (no per-chunk ACT_TABLE_LOAD thrash)."""

    _KEEP_TABLES = ("natural_log_exp_and_others", "sigmoid_and_others")

    def insert_act_table_loads(self):
        import bass_rust as _bass_rust
        from concourse.hw_specs import get_activation_tables
        import concourse.mybir as _mb
        has_activation = any(
            isinstance(i, _mb.InstActivation)
            for b in self.main_func.blocks
            for i in b.instructions
        )
        if not has_activation:
            return
        # act_func_set_id is positional in act_info.json order: keep every
        # entry but empty the unwanted ones so the chooser can't pick them.
        all_tables = get_activation_tables(self.m.arch)
        tables = [(k, (v if k in self._KEEP_TABLES else set()))
                  for k, v in all_tables.items()]
        _bass_rust.insert_act_table_loads(self, tables)


def _round_fp32r(a):
    """Round fp32 to the PE's fp32r grid (1s+8e+11m, RNE) on the host."""
    b = np.ascontiguousarray(a, dtype=np.float32).view(np.uint32)
    lsb = (b >> 12) & np.uint32(1)
    b2 = ((b + np.uint32(0x7FF) + lsb) & np.uint32(0xFFFFF000)).astype(np.uint32)
    return b2.view(np.float32)


def _build(bc, mm_dt):
    """Build the per-core Bass module for bc batch rows per core."""
    nch = bc // 128
    MMT = mm_dt
    if USE_SCANMUL:
        scan_mul, cumsum_op = _register_scan_mul()
    else:
        scan_mul = cumsum_op = None
    nc = _Bacc("TRN2", target_bir_lowering=False, debug=False,
               enable_asserts=False)

    def din(name, shape, dt=F32):
        return nc.dram_tensor(name, list(shape), dt, kind="ExternalInput").ap()

    pred = din("pred", (bc, F))
    ctxm = din("ctx", (bc, C))
    w_in = din("w_in", (F, H), MMT)
    wc_in = din("wc_in", (C, H), MMT)
    wb1 = din("wb1", (NBLOCKS, H, H), MMT)
    wb2 = din("wb2", (NBLOCKS, H, H), MMT)
    wcb = din("wcb", (NBLOCKS, C, H), MMT)
    w_out = din("w_out", (H, F * MULT), MMT)
    b1 = din("b1", (H,))
    bb1 = din("bb1", (NBLOCKS, H))
    bb2 = din("bb2", (NBLOCKS, H))
    bcb = din("bcb", (NBLOCKS, H))
    ident = din("ident", (128, 128))
    k1c = din("k1c", (NB - 1,))
    out_d = nc.dram_tensor("out", [bc], F32, kind="ExternalOutput").ap()

    AX = mybir.AxisListType
    OP = mybir.AluOpType
    ACTF = mybir.ActivationFunctionType

    def bcast(ap2d, n):
        """[P, M] AP -> [P, M, n] with stride-0 inner (broadcast along bins)."""
        return bass.AP(tensor=ap2d.tensor, offset=ap2d.offset,
                       ap=list(ap2d.ap) + [[0, n]])

    def pbcast(ap1d, p, n):
        """[n] DRAM AP -> [p, n] with stride-0 partitions (for DMA)."""
        return bass.AP(tensor=ap1d.tensor, offset=ap1d.offset,
                       ap=[[0, p]] + list(ap1d.ap))

    with tile.TileContext(nc) as tc, ExitStack() as ctx:
        const = ctx.enter_context(tc.tile_pool(name="const", bufs=1))
        persist = ctx.enter_context(tc.tile_pool(name="persist", bufs=1))

        ident_t = const.tile([128, 128], F32)
        nc.sync.dma_start(out=ident_t[:], in_=ident)
        k1_t = const.tile([128, NB - 1], F32)
        nc.sync.dma_start(out=k1_t[:], in_=pbcast(k1c, 128, NB - 1))
        one_t = const.tile([128, 1], F32)
        nc.vector.memset(one_t[:], 1.0)
        mb_t = const.tile([128, 1], F32)
        nc.vector.memset(mb_t[:], MIN_BIN)

        # persistent activations
        t_t = [persist.tile([128, bc], MMT, tag=f"t{k}", name=f"t{k}") for k in range(KH)]
        halfprod = persist.tile([128, nch, 2], F32)

        # ---------------- Phase A: transposes + MADE trunk ----------------
        with tc.tile_pool(name="pa", bufs=1) as pa, \
             tc.tile_pool(name="paw", bufs=2) as paw, \
             tc.tile_pool(name="pat", bufs=2) as pat, \
             ExitStack() as actx:

            ctx_T = [pa.tile([128, bc], MMT, tag=f"ctxT{k}", name=f"ctxT{k}") for k in range(KH)]
            x_T = pa.tile([64, bc], MMT)

            w_in_t = pa.tile([64, H], MMT)
            nc.sync.dma_start(out=w_in_t[:], in_=w_in)
            wc_in_t = [pa.tile([128, H], MMT, tag=f"wci{k}", name=f"wci{k}") for k in range(KH)]
            for k in range(KH):
                nc.sync.dma_start(out=wc_in_t[k][:],
                                  in_=wc_in[k * 128:(k + 1) * 128, :])
            b1_t = pa.tile([128, KH], F32)
            nc.sync.dma_start(out=b1_t[:],
                              in_=b1.rearrange("(m p) -> p m", p=128))
            bb1_t = pa.tile([128, NBLOCKS, KH], F32)
            bb2_t = pa.tile([128, NBLOCKS, KH], F32)
            bcb_t = pa.tile([128, NBLOCKS, KH], F32)
            for tt_, src in ((bb1_t, bb1), (bb2_t, bb2), (bcb_t, bcb)):
                nc.sync.dma_start(out=tt_[:],
                                  in_=src.rearrange("i (m p) -> p i m", p=128))

            # transpose ctx and pred chunks on the PE
            pst_cm = tc.tile_pool(name="pst", bufs=2, space="PSUM")
            psa_pool = [None]
            pst = pst_cm.__enter__()
            for c in range(nch):
                ld = pat.tile([128, C], F32, tag="ctxld", name="ctxld")
                nc.sync.dma_start(out=ld[:], in_=ctxm[c * 128:(c + 1) * 128, :])
                for k in range(KH):
                    ps = pst.tile([128, 128], F32, tag="tp", name="tp")
                    nc.tensor.transpose(ps[:], ld[:, k * 128:(k + 1) * 128],
                                        ident_t[:])
                    nc.scalar.activation(out=ctx_T[k][:, c * 128:(c + 1) * 128],
                                         in_=ps[:], func=ACTF.Copy)
                pld = pat.tile([128, F], F32, tag="predld", name="predld")
                nc.sync.dma_start(out=pld[:], in_=pred[c * 128:(c + 1) * 128, :])
                ps = pst.tile([64, 128], F32, tag="tpp", name="tpp")
                nc.tensor.transpose(ps[:], pld[:], ident_t[:])
                nc.scalar.activation(out=x_T[:, c * 128:(c + 1) * 128],
                                     in_=ps[:], func=ACTF.Copy)

            pst_cm.__exit__(None, None, None)
            psa = actx.enter_context(tc.tile_pool(name="psa", bufs=2, space="PSUM"))
            bsw = min(512, bc)
            nbs = bc // bsw
            # input projection: t = x @ (W_in*M) + ctx @ Wc_in + b1
            for bs in range(nbs):
                bsl = slice(bs * bsw, (bs + 1) * bsw)
                for m in range(KH):
                    msl = slice(m * 128, (m + 1) * 128)
                    ps = psa.tile([128, bsw], F32, tag="mm", name="mm")
                    nc.tensor.matmul(ps[:], w_in_t[:, msl],
                                     x_T[:, bsl],
                                     start=True, stop=False)
                    for k in range(KH):
                        nc.tensor.matmul(ps[:], wc_in_t[k][:, msl],
                                         ctx_T[k][:, bsl],
                                         start=False, stop=(k == KH - 1))
                    nc.scalar.activation(out=t_t[m][:, bsl], in_=ps[:],
                                         func=ACTF.Identity, bias=b1_t[:, m:m + 1])

            # residual blocks
            for i in range(NBLOCKS):
                wb1_i = [paw.tile([128, H], MMT, tag=f"wb1_{k}", name=f"wb1_{k}") for k in range(KH)]
                wb2_i = [paw.tile([128, H], MMT, tag=f"wb2_{k}", name=f"wb2_{k}") for k in range(KH)]
                wcb_i = [paw.tile([128, H], MMT, tag=f"wcb_{k}", name=f"wcb_{k}") for k in range(KH)]
                for k in range(KH):
                    ksl = slice(k * 128, (k + 1) * 128)
                    nc.sync.dma_start(out=wb1_i[k][:], in_=wb1[i, ksl, :])
                    nc.sync.dma_start(out=wb2_i[k][:], in_=wb2[i, ksl, :])
                    nc.sync.dma_start(out=wcb_i[k][:], in_=wcb[i, ksl, :])
                for bs in range(nbs):
                    bsl = slice(bs * bsw, (bs + 1) * bsw)
                    h1t = pat.tile([128, KH, bsw], MMT, tag="h1t", name="h1t")
                    for k in range(KH):
                        nc.scalar.activation(out=h1t[:, k, :],
                                             in_=t_t[k][:, bsl],
                                             func=ACTF.Relu)
                    h1 = [h1t[:, k, :] for k in range(KH)]
                    h2t = pat.tile([128, KH, bsw], MMT, tag="h2t", name="h2t")
                    h2 = [h2t[:, k, :] for k in range(KH)]
                    for m in range(KH):
                        msl = slice(m * 128, (m + 1) * 128)
                        ps = psa.tile([128, bsw], F32, tag="mm1", name="mm1")
                        for k in range(KH):
                            nc.tensor.matmul(ps[:], wb1_i[k][:, msl],
                                             h1[k],
                                             start=(k == 0), stop=(k == KH - 1))
                        nc.scalar.activation(out=h2[m], in_=ps[:],
                                             func=ACTF.Relu,
                                             bias=bb1_t[:, i, m:m + 1])
                    for m in range(KH):
                        msl = slice(m * 128, (m + 1) * 128)
                        ps2 = psa.tile([128, bsw], F32, tag="mm2", name="mm2")
                        for k in range(KH):
                            nc.tensor.matmul(ps2[:], wb2_i[k][:, msl],
                                             h2[k],
                                             start=(k == 0), stop=(k == KH - 1))
                        ps3 = psa.tile([128, bsw], F32, tag="mm3", name="mm3")
                        for k in range(KH):
                            nc.tensor.matmul(ps3[:], wcb_i[k][:, msl],
                                             ctx_T[k][:, bsl],
                                             start=(k == 0), stop=(k == KH - 1))
                        g = pat.tile([128, bsw], F32, tag="g", name="g")
                        nc.scalar.activation(out=g[:], in_=ps3[:], func=ACTF.Sigmoid,
                                             bias=bcb_t[:, i, m:m + 1])
                        v = pat.tile([128, bsw], F32, tag="v", name="v")
                        nc.vector.scalar_tensor_tensor(
                            out=v[:], in0=ps2[:], scalar=bb2_t[:, i, m:m + 1],
                            in1=g[:], op0=OP.add, op1=OP.mult)
                        nc.gpsimd.tensor_tensor(out=t_t[m][:, bsl],
                                                 in0=t_t[m][:, bsl], in1=v[:],
                                                 op=OP.add)

        # ---------------- Phase B: output GEMM + spline ----------------
        GRP = min(8, nch)
        assert nch % GRP == 0
        with tc.tile_pool(name="pb", bufs=1) as pb, \
             tc.tile_pool(name="spl", bufs=2) as spl, \
             tc.tile_pool(name="grp", bufs=2) as grp, \
             tc.tile_pool(name="psb", bufs=3, space="PSUM") as psb:

            TS = nc.vector.tensor_scalar
            TT = nc.vector.tensor_tensor

            def tscopy(dst, srcap):
                TS(out=dst, in0=srcap, scalar1=0.0, scalar2=None, op0=OP.add)

            for half in range(2):
                wo_t = [pb.tile([128, WOH], MMT, tag=f"wo{k}", name=f"wo{k}")
                        for k in range(KH)]
                for k in range(KH):
                    nc.sync.dma_start(
                        out=wo_t[k][:],
                        in_=w_out[k * 128:(k + 1) * 128,
                                  half * WOH:(half + 1) * WOH])
                for gidx in range(nch // GRP):
                    def gt(nm):
                        return grp.tile([128, GRP, FH], F32, tag=nm, name=nm)
                    gGL = gt("gGL"); gCR = gt("gCR"); gIDX = gt("gIDX")
                    gSH = gt("gSH"); gX = gt("gX"); gEW0 = gt("gEW0")
                    gEH0 = gt("gEH0"); gD0 = gt("gD0"); gD1 = gt("gD1")
                    gRall = grp.tile([128, GRP, 6, FH], F32, tag="gRall",
                                     name="gRall")

                    for gi in range(GRP):
                        c = gidx * GRP + gi
                        csl = slice(c * 128, (c + 1) * 128)
                        nc.sync.dma_start(
                            out=gX[:, gi, :],
                            in_=pred[csl, half * FH:(half + 1) * FH])
                        EW = spl.tile([128, FH, NB], F32, tag="EW", name="EW")
                        EH = spl.tile([128, FH, NB], F32, tag="EH", name="EH")
                        ED = spl.tile([128, FH, NB + 1], F32, tag="ED", name="ED")
                        for n in range(2):
                            ps = psb.tile([128, 4, 512], F32, tag="pp",
                                          name="pp", bufs=2)
                            for j in range(4):
                                nsl = slice((n * 4 + j) * 364,
                                            (n * 4 + j + 1) * 364)
                                for k in range(KH):
                                    nc.tensor.matmul(
                                        ps[:, j, 0:364],
                                        t_t[k][:, csl],
                                        wo_t[k][:, nsl],
                                        start=(k == 0), stop=(k == KH - 1))
                            psv = bass.AP(tensor=ps[:].tensor,
                                          offset=ps[:].offset,
                                          ap=[ps[:].ap[0], [512, 4], [MULT, 4],
                                              [1, MULT]])
                            fsl = slice(n * 16, (n + 1) * 16)
                            nc.scalar.activation(
                                out=EW[:, fsl, :].rearrange(
                                    "p (a f) n -> p a f n", a=4),
                                in_=psv[:, :, :, 0:NB],
                                func=ACTF.Exp, scale=SCALE)
                            nc.scalar.activation(
                                out=EH[:, fsl, :].rearrange(
                                    "p (a f) n -> p a f n", a=4),
                                in_=psv[:, :, :, NB:2 * NB],
                                func=ACTF.Exp, scale=SCALE)
                            nc.scalar.activation(
                                out=ED[:, fsl, :].rearrange(
                                    "p (a f) n -> p a f n", a=4),
                                in_=psv[:, :, :, 2 * NB:MULT],
                                func=ACTF.Exp)
                        # D = softplus(ud) = ln(exp(ud) + 1), in place over ED
                        D = ED
                        nc.scalar.activation(
                            out=D[:].rearrange("p f n -> p (f n)"),
                            in_=ED[:].rearrange("p f n -> p (f n)"),
                            func=ACTF.Ln, bias=one_t[:])
                        # chained scan of EW across the whole half
                        Gg = spl.tile([128, FH, NB], F32, tag="Gg", name="Gg", bufs=1)
                        if USE_SCANMUL:
                            nc.vector._custom_dve(
                                cumsum_op,
                                out=Gg[:].rearrange("p f n -> p (f n)"),
                                in0=EW[:].rearrange("p f n -> p (f n)"))
                        else:
                            nc.vector.tensor_tensor_scan(
                                out=Gg[:].rearrange("p f n -> p (f n)"),
                                data0=EW[:].rearrange("p f n -> p (f n)"),
                                data1=EW[:].rearrange("p f n -> p (f n)"),
                                initial=0.0, op0=OP.add, op1=OP.bypass)
                        Gl = bass.AP(tensor=Gg[:].tensor,
                                     offset=Gg[:].offset + NB - 1,
                                     ap=[Gg[:].ap[0], [NB, FH]])
                        tscopy(gGL[:, gi, :], Gl)
                        Sw = spl.tile([128, FH], F32, tag="Sw", name="Sw")
                        tscopy(Sw[:, 0:1], Gl[:, 0:1])
                        TT(out=Sw[:, 1:FH], in0=Gl[:, 1:FH],
                           in1=Gl[:, 0:FH - 1], op=OP.subtract)
                        Rw = spl.tile([128, FH], F32, tag="Rw", name="Rw")
                        nc.vector.reciprocal(out=Rw[:], in_=Sw[:])
                        CR = spl.tile([128, FH], F32, tag="CR", name="CR")
                        TS(out=CR[:], in0=Rw[:], scalar1=CFREE, scalar2=None,
                           op0=OP.mult)
                        tscopy(gCR[:, gi, :], CR[:])
                        xp = spl.tile([128, FH], F32, tag="xp", name="xp")
                        tscopy(xp[:, 0:1], gX[:, gi, 0:1])
                        P2 = spl.tile([128, FH], F32, tag="P2", name="P2")
                        TT(out=P2[:, 1:FH], in0=Gl[:, 0:FH - 1],
                           in1=CR[:, 1:FH], op=OP.mult)
                        TT(out=xp[:, 1:FH], in0=gX[:, gi, 1:FH],
                           in1=P2[:, 1:FH], op=OP.add)
                        # masks
                        # XK = x' - K1 is off the critical chain (no Gg dep)
                        XK = spl.tile([128, FH, NB - 1], F32, tag="XK",
                                      name="XK", bufs=1)
                        k1b = bass.AP(tensor=k1_t[:].tensor,
                                      offset=k1_t[:].offset,
                                      ap=[k1_t[:].ap[0], [0, FH], [1, NB - 1]])
                        nc.gpsimd.tensor_tensor(out=XK[:],
                                                in0=bcast(xp[:], NB - 1),
                                                in1=k1b, op=OP.subtract)
                        ENm = spl.tile([128, FH, NB], F32, tag="ENm",
                                       name="ENm", bufs=1)
                        nc.gpsimd.tensor_tensor(out=ENm[:], in0=Gg[:],
                                                in1=bcast(CR[:], NB),
                                                op=OP.mult)
                        u = spl.tile([128, FH, NB - 1], F32, tag="u", name="u")
                        TT(out=u[:], in0=XK[:], in1=ENm[:, :, 0:NB - 1],
                           op=OP.is_ge)
                        nc.vector.tensor_reduce(out=gIDX[:, gi, :], in_=u[:],
                                                axis=AX.X, op=OP.add)
                        nc.vector.tensor_reduce(out=gSH[:, gi, :], in_=EH[:],
                                                axis=AX.X, op=OP.add)
                        dD = spl.tile([128, FH, NB], F32, tag="dD", name="dD", bufs=1)
                        nc.gpsimd.tensor_tensor(out=dD[:],
                                                in0=D[:, :, 1:NB + 1],
                                                in1=D[:, :, 0:NB],
                                                op=OP.subtract)
                        nc.scalar.activation(
                            out=gEW0[:, gi, :],
                            in_=bass.AP(tensor=EW[:].tensor,
                                        offset=EW[:].offset,
                                        ap=[EW[:].ap[0], [NB, FH]]),
                            func=ACTF.Copy)
                        nc.scalar.activation(
                            out=gEH0[:, gi, :],
                            in_=bass.AP(tensor=EH[:].tensor,
                                        offset=EH[:].offset,
                                        ap=[EH[:].ap[0], [NB, FH]]),
                            func=ACTF.Copy)
                        nc.scalar.activation(
                            out=gD0[:, gi, :],
                            in_=bass.AP(tensor=D[:].tensor, offset=D[:].offset,
                                        ap=[D[:].ap[0], [NB + 1, FH]]),
                            func=ACTF.Copy)
                        nc.scalar.activation(
                            out=gD1[:, gi, :],
                            in_=bass.AP(tensor=D[:].tensor,
                                        offset=D[:].offset + 1,
                                        ap=[D[:].ap[0], [NB + 1, FH]]),
                            func=ACTF.Copy)
                        streams = (EW[:, :, 0:NB - 1], EW[:, :, 1:NB],
                                   EH[:, :, 0:NB - 1], EH[:, :, 1:NB],
                                   dD[:, :, 0:NB - 1], dD[:, :, 1:NB])
                        Rbig = spl.tile([128, 6, FH, NB - 1], F32,
                                        tag="Rbig", name="Rbig", bufs=1)
                        for i_s, tsl in enumerate(streams):
                            if USE_SCANMUL:
                                nc.vector._custom_dve(scan_mul,
                                                      out=Rbig[:, i_s, :, :],
                                                      in0=u[:], in1=tsl)
                            else:
                                TT(out=Rbig[:, i_s, :, :], in0=u[:], in1=tsl,
                                   op=OP.mult)
                                nc.vector.tensor_reduce(
                                    out=gRall[:, gi, i_s, :],
                                    in_=Rbig[:, i_s, :, :],
                                    axis=AX.X, op=OP.add)
                        if USE_SCANMUL:
                            # one extraction for all six gathers
                            Rl6 = bass.AP(tensor=Rbig[:].tensor,
                                          offset=Rbig[:].offset + NB - 2,
                                          ap=[Rbig[:].ap[0], [FH * (NB - 1), 6],
                                              [NB - 1, FH]])
                            tscopy(gRall[:, gi, :, :], Rl6)

                    # ---- grouped small chain on [128, GRP, FH] tiles ----
                    def g2t(nm):
                        return grp.tile([128, GRP, FH], F32, tag=nm, name=nm,
                                        bufs=1)
                    if USE_SCANMUL:
                        gdall = grp.tile([128, GRP, 6, FH], F32, tag="gdall",
                                         name="gdall", bufs=1)
                        TT(out=gdall[:, :, :, 1:FH],
                           in0=gRall[:, :, :, 1:FH],
                           in1=gRall[:, :, :, 0:FH - 1], op=OP.subtract)
                        tscopy(gdall[:, :, :, 0:1], gRall[:, :, :, 0:1])
                    else:
                        gdall = gRall
                    g1 = gdall[:, :, 0, :]
                    g2_ = gdall[:, :, 1, :]
                    g3 = gdall[:, :, 2, :]
                    g4 = gdall[:, :, 3, :]
                    g5 = gdall[:, :, 4, :]
                    g6 = gdall[:, :, 5, :]
                    t1 = g2t("t1")
                    nc.scalar.activation(out=t1[:], in_=gIDX[:],
                                         func=ACTF.Copy, scale=MIN_BIN)
                    incw = g2t("incw")
                    TT(out=incw[:], in0=gCR[:], in1=g1, op=OP.mult)
                    TT(out=incw[:], in0=incw[:], in1=t1[:], op=OP.add)
                    ewi = g2t("ewi")
                    TT(out=ewi[:], in0=g2_, in1=g1, op=OP.subtract)
                    TT(out=ewi[:], in0=ewi[:], in1=gEW0[:], op=OP.add)
                    inw = g2t("inw")
                    TT(out=inw[:], in0=gCR[:], in1=ewi[:], op=OP.mult)
                    nc.scalar.activation(out=inw[:], in_=inw[:],
                                         func=ACTF.Identity, bias=mb_t[:])
                    rw_ = g2t("rw_")
                    nc.vector.reciprocal(out=rw_[:], in_=inw[:])
                    th = g2t("th")
                    TT(out=th[:], in0=gX[:], in1=incw[:], op=OP.subtract)
                    TT(out=th[:], in0=th[:], in1=rw_[:], op=OP.mult)
                    gRH = g2t("gRH")
                    nc.vector.reciprocal(out=gRH[:], in_=gSH[:])
                    gCH = g2t("gCH")
                    TS(out=gCH[:], in0=gRH[:], scalar1=CFREE, scalar2=None,
                       op0=OP.mult)
                    inch = g2t("inch")
                    TT(out=inch[:], in0=gCH[:], in1=g3, op=OP.mult)
                    TT(out=inch[:], in0=inch[:], in1=t1[:], op=OP.add)
                    ehi = g2t("ehi")
                    TT(out=ehi[:], in0=g4, in1=g3, op=OP.subtract)
                    TT(out=ehi[:], in0=ehi[:], in1=gEH0[:], op=OP.add)
                    inh = g2t("inh")
                    TT(out=inh[:], in0=gCH[:], in1=ehi[:], op=OP.mult)
                    nc.scalar.activation(out=inh[:], in_=inh[:],
                                         func=ACTF.Identity, bias=mb_t[:])
                    ind = g2t("ind")
                    nc.vector.scalar_tensor_tensor(out=ind[:], in0=g5,
                                                   scalar=MIN_DERIV,
                                                   in1=gD0[:], op0=OP.add,
                                                   op1=OP.add)
                    indp = g2t("indp")
                    nc.vector.scalar_tensor_tensor(out=indp[:], in0=g6,
                                                   scalar=MIN_DERIV,
                                                   in1=gD1[:], op0=OP.add,
                                                   op1=OP.add)
                    dl = g2t("dl")
                    TT(out=dl[:], in0=inh[:], in1=rw_[:], op=OP.mult)
                    om = g2t("om")
                    nc.scalar.activation(out=om[:], in_=th[:],
                                         func=ACTF.Identity, bias=one_t[:],
                                         scale=-1.0)
                    ttv = g2t("ttv")
                    TT(out=ttv[:], in0=th[:], in1=om[:], op=OP.mult)
                    th2 = g2t("th2")
                    nc.scalar.activation(out=th2[:], in_=th[:],
                                         func=ACTF.Square)
                    na = g2t("na")
                    TT(out=na[:], in0=dl[:], in1=th2[:], op=OP.mult)
                    nb_ = g2t("nb_")
                    TT(out=nb_[:], in0=ind[:], in1=ttv[:], op=OP.mult)
                    TT(out=na[:], in0=na[:], in1=nb_[:], op=OP.add)
                    TT(out=na[:], in0=na[:], in1=inh[:], op=OP.mult)
                    s1_ = g2t("s1_")
                    TT(out=s1_[:], in0=ind[:], in1=indp[:], op=OP.add)
                    nc.vector.scalar_tensor_tensor(out=s1_[:], in0=dl[:],
                                                   scalar=-2.0, in1=s1_[:],
                                                   op0=OP.mult, op1=OP.add)
                    TT(out=s1_[:], in0=s1_[:], in1=ttv[:], op=OP.mult)
                    TT(out=s1_[:], in0=s1_[:], in1=dl[:], op=OP.add)
                    rden = g2t("rden")
                    nc.vector.reciprocal(out=rden[:], in_=s1_[:])
                    cdf = g2t("cdf")
                    TT(out=cdf[:], in0=na[:], in1=rden[:], op=OP.mult)
                    TT(out=cdf[:], in0=cdf[:], in1=inch[:], op=OP.add)
                    # product over the 32 features of this half
                    TT(out=cdf[:, :, 0:16], in0=cdf[:, :, 0:16],
                       in1=cdf[:, :, 16:32], op=OP.mult)
                    TT(out=cdf[:, :, 0:8], in0=cdf[:, :, 0:8],
                       in1=cdf[:, :, 8:16], op=OP.mult)
                    TT(out=cdf[:, :, 0:4], in0=cdf[:, :, 0:4],
                       in1=cdf[:, :, 4:8], op=OP.mult)
                    TT(out=cdf[:, :, 0:2], in0=cdf[:, :, 0:2],
                       in1=cdf[:, :, 2:4], op=OP.mult)
                    TT(out=halfprod[:, gidx * GRP:(gidx + 1) * GRP,
                                    half:half + 1],
                       in0=cdf[:, :, 0:1], in1=cdf[:, :, 1:2], op=OP.mult)

            fp = persist.tile([128, nch], F32)
            nc.vector.tensor_tensor(
                out=fp[:],
                in0=halfprod[:, :, 0:1].rearrange("p c h -> p (c h)"),
                in1=halfprod[:, :, 1:2].rearrange("p c h -> p (c h)"),
                op=OP.mult)
            nc.sync.dma_start(out=out_d.rearrange("(c p) -> p c", p=128),
                              in_=fp[:])

    nc.compile()
    return nc


def _prep_shared(W_in, b_in, Wc_in, bc_in, Wb1, bb1, Wb2, bb2, Wcb, bcb,
                 W_out, b_out, mm_dt):
    m_in, m_hh, m_out = _masks()
    assert not np.any(b_out), "nonzero b_out not supported by this kernel"
    rnd = _round_fp32r if mm_dt == mybir.dt.float32r else (
        lambda a: np.ascontiguousarray(a, dtype=np.float32))
    shared = {
        "w_in": rnd(W_in * m_in),
        "wc_in": rnd(Wc_in),
        "wb1": rnd(Wb1 * m_hh[None]),
        "wb2": rnd(Wb2 * m_hh[None]),
        "wcb": rnd(Wcb),
        "w_out": rnd(W_out * m_out),
        "b1": np.ascontiguousarray((b_in + bc_in).astype(np.float32)),
        "bb1": np.ascontiguousarray(bb1.astype(np.float32)),
        "bb2": np.ascontiguousarray(bb2.astype(np.float32)),
        "bcb": np.ascontiguousarray(bcb.astype(np.float32)),
        "ident": np.eye(128, dtype=np.float32),
        "k1c": (MIN_BIN * np.arange(1, NB)).astype(np.float32),
    }
    return shared


def kernel(predicates, contexts, W_in, b_in, Wc_in, bc_in, Wb1, bb1, Wb2, bb2,
           Wcb, bcb, W_out, b_out):
    global LAST_RESULTS
    predicates = np.asarray(predicates, dtype=np.float32)
    contexts = np.asarray(contexts, dtype=np.float32)
    bc = predicates.shape[0] // NCORES
    key = (bc, MM_DT, USE_SCANMUL)
    if key not in _CACHE:
        _CACHE[key] = _build(bc, MM_DT)
    nc = _CACHE[key]
    shared = _prep_shared(W_in, b_in, Wc_in, bc_in, Wb1, bb1, Wb2, bb2,
                          Wcb, bcb, W_out, b_out, MM_DT)
    in_maps = []
    for cid in range(NCORES):
        sl = slice(cid * bc, (cid + 1) * bc)
        m = dict(shared)
        m["pred"] = np.ascontiguousarray(predicates[sl])
        m["ctx"] = np.ascontiguousarray(contexts[sl])
        in_maps.append(m)
    res = run_bass_kernel_spmd(nc, in_maps, core_ids=list(range(NCORES)),
                               trace=TRACE)
    LAST_RESULTS = res
    return np.concatenate([res.results[i]["out"] for i in range(NCORES)])



# revision 5
# speedup vs baseline: 1.0635x; 1.0635x over previous
"""Trainium2 Bass kernel for nn_AutoregressiveCDF (MADE + rational-quadratic
spline CDF, product over features).

Strategy: pure data-parallel over 8 NeuronCores (batch 16384 -> 8 x 2048),
weights replicated.  Per core:
  Phase A: transpose predicates/contexts via PE, run the MADE trunk as
           hidden-on-partition GEMMs (fp32r on the PE), activations on ACT.
  Phase B: output GEMM t @ W_out in [batch-part, feature-free] orientation,
           PSUM consumed directly by ACT exp; spline evaluated in the
           *normalized* domain: widths scaled by CFREE/Sw so each feature's
           edge span is exactly 1.0, making the chained running edge value
           at feature f equal f + local_edge.  A fused scan-compare custom
           DVE op then yields the bin mask u in one pass (compare against
           x + f), and six fused scan-MAC ops produce the gathered spline
           parameters (prefix sums at the bin index via segment-boundary
           diffs).  The min-bin affine is folded into the scans (imm2), so
           no bin-index tensor, searchsorted gather, or edge tensor is ever
           materialized.
"""

import numpy as np
from contextlib import ExitStack

import concourse.bass as bass
import concourse.bacc as bacc
import concourse.tile as tile
from concourse import mybir
from concourse.bass_utils import run_bass_kernel_spmd

F32 = mybir.dt.float32

# problem sizes (hardcoded per contract)
B, F, H, C = 16384, 64, 512, 512
NB = 30
MULT = 3 * NB + 1            # 91
NBLOCKS = 3
NCORES = 8
MIN_BIN = 1e-3
MIN_DERIV = 1e-3
CFREE = float(1.0 - MIN_BIN * NB)         # softmax mass after min-bin affine
SCALE = float(np.float32(1.0 / np.sqrt(H)))
FH = F // 2                  # features per half (32)
WOH = FH * MULT              # 2912 W_out cols per half
KH = H // 128                # 4 hidden chunks

# knobs (test.py may override module globals before calling kernel())
MM_DT = mybir.dt.float32r    # PE dtype: float32r (fast) or float32 (safe)
TRACE = False
LAST_RESULTS = None          # BassKernelResults of the most recent run

_CACHE = {}


def _masks():
    d_in = np.arange(1, F + 1)
    d_h = np.arange(H) % max(1, F - 1) + min(1, F - 1)
    m_in = (d_h[None, :] >= d_in[:, None]).astype(np.float32)
    m_hh = (d_h[None, :] >= d_h[:, None]).astype(np.float32)
    d_out = np.repeat(d_in, MULT)
    m_out = (d_out[None, :] > d_h[:, None]).astype(np.float32)
    return m_in, m_hh, m_out


def _scanmac_ref(in0, in1, s0, s1, imm2):
    a = np.asarray(in0, np.float32).reshape(np.asarray(in0).shape[0], -1)
    b = np.asarray(in1, np.float32).reshape(a.shape)
    return np.cumsum(a * (b + np.float32(s0)), axis=1,
                     dtype=np.float32).reshape(np.asarray(in0).shape)


def _scancmp_ref(in0, in1, s0, s1, imm2):
    a = np.asarray(in0, np.float32).reshape(np.asarray(in0).shape[0], -1)
    t = np.asarray(in1, np.float32).reshape(a.shape)
    s = np.cumsum(a + np.float32(s0), axis=1, dtype=np.float32)
    return (t >= s).astype(np.float32).reshape(np.asarray(in0).shape)


def _register_spline_ops():
    """Register the two fused DVE ops the spline needs:

    SCAN_MAC_ANT: out = cumsum(in0 * (in1 + s0))   (chained masked MAC)
    SCANCMP_ANT:  out = (in1 >= cumsum(in0 + s0))  (bin-search mask)
    """
    import concourse.dve_ops as dve_ops
    from concourse.dve_spec import Spec, Src0, Src1, C0, scan, AluOp, lower
    from concourse.dve_uop import DveOpSpec
    have = {op.name: op for op in dve_ops.OPS}
    if "SCAN_MAC_ANT" in have and "SCANCMP_ANT" in have:
        return have["SCAN_MAC_ANT"], have["SCANCMP_ANT"]

    def reg(name, spec):
        row = max(dve_ops._SUB_OPCODE_FOR_NAME.values()) + 1
        assert row < 0x20
        shas = {}
        for ver in ("v3", "v4"):
            u = lower(spec, ver=ver)
            shas[ver] = DveOpSpec(name=name, opcode=row, uops=u,
                                  rd1_en=True).sha(ver)
        op = dve_ops.DveOp(name, spec, subdim=False, uops_sha=shas)
        dve_ops.OPS.append(op)
        dve_ops.CUSTOM_DVE_SPECS[name] = spec
        dve_ops._SUB_OPCODE_FOR_NAME[name] = row
        return op

    mac = reg("SCAN_MAC_ANT",
              Spec(body=scan(AluOp.ADD, Src0 * (Src1 + C0)),
                   reference=_scanmac_ref))
    cmp_ = reg("SCANCMP_ANT",
               Spec(body=scan(AluOp.ADD, Src0 + C0) <= Src1,
                    reference=_scancmp_ref))
    return mac, cmp_


class _Bacc(bacc.Bacc):
    """Bacc with a trimmed activation-table list so Exp and Ln share one
    table (no per-chunk ACT_TABLE_LOAD thrash)."""

    _KEEP_TABLES = ("natural_log_exp_and_others", "sigmoid_and_others")

    def insert_act_table_loads(self):
        import bass_rust as _bass_rust
        from concourse.hw_specs import get_activation_tables
        import concourse.mybir as _mb
        has_activation = any(
            isinstance(i, _mb.InstActivation)
            for b in self.main_func.blocks
            for i in b.instructions
        )
        if not has_activation:
            return
        # act_func_set_id is positional in act_info.json order: keep every
        # entry but empty the unwanted ones so the chooser can't pick them.
        all_tables = get_activation_tables(self.m.arch)
        tables = [(k, (v if k in self._KEEP_TABLES else set()))
                  for k, v in all_tables.items()]
        _bass_rust.insert_act_table_loads(self, tables)


def _round_fp32r(a):
    """Round fp32 to the PE's fp32r grid (1s+8e+11m, RNE) on the host."""
    b = np.ascontiguousarray(a, dtype=np.float32).view(np.uint32)
    lsb = (b >> 12) & np.uint32(1)
    b2 = ((b + np.uint32(0x7FF) + lsb) & np.uint32(0xFFFFF000)).astype(np.uint32)
    return b2.view(np.float32)


def _build(bc, mm_dt):
    """Build the per-core Bass module for bc batch rows per core."""
    nch = bc // 128
    MMT = mm_dt
    scan_mac, scancmp = _register_spline_ops()
    nc = _Bacc("TRN2", target_bir_lowering=False, debug=False,
               enable_asserts=False)

    def din(name, shape, dt=F32):
        return nc.dram_tensor(name, list(shape), dt, kind="ExternalInput").ap()

    pred = din("pred", (bc, F))
    ctxm = din("ctx", (bc, C))
    w_in = din("w_in", (F, H), MMT)
    wc_in = din("wc_in", (C, H), MMT)
    wb1 = din("wb1", (NBLOCKS, H, H), MMT)
    wb2 = din("wb2", (NBLOCKS, H, H), MMT)
    wcb = din("wcb", (NBLOCKS, C, H), MMT)
    w_out = din("w_out", (H, F * MULT), MMT)
    b1 = din("b1", (H,))
    bb1 = din("bb1", (NBLOCKS, H))
    bb2 = din("bb2", (NBLOCKS, H))
    bcb = din("bcb", (NBLOCKS, H))
    ident = din("ident", (128, 128))
    fcon = din("fcon", (FH,))
    out_d = nc.dram_tensor("out", [bc], F32, kind="ExternalOutput").ap()

    AX = mybir.AxisListType
    OP = mybir.AluOpType
    ACTF = mybir.ActivationFunctionType

    def bcast(ap2d, n):
        """[P, M] AP -> [P, M, n] with stride-0 inner (broadcast along bins)."""
        return bass.AP(tensor=ap2d.tensor, offset=ap2d.offset,
                       ap=list(ap2d.ap) + [[0, n]])

    def pbcast(ap1d, p, n):
        """[n] DRAM AP -> [p, n] with stride-0 partitions (for DMA)."""
        return bass.AP(tensor=ap1d.tensor, offset=ap1d.offset,
                       ap=[[0, p]] + list(ap1d.ap))

    with tile.TileContext(nc) as tc, ExitStack() as ctx:
        const = ctx.enter_context(tc.tile_pool(name="const", bufs=1))
        persist = ctx.enter_context(tc.tile_pool(name="persist", bufs=1))

        ident_t = const.tile([128, 128], F32)
        nc.sync.dma_start(out=ident_t[:], in_=ident)
        fc_t = const.tile([128, FH], F32)
        nc.sync.dma_start(out=fc_t[:], in_=pbcast(fcon, 128, FH))
        one_t = const.tile([128, 1], F32)
        nc.vector.memset(one_t[:], 1.0)

        # persistent activations
        t_t = [persist.tile([128, bc], MMT, tag=f"t{k}", name=f"t{k}") for k in range(KH)]
        halfprod = persist.tile([128, nch, 2], F32)

        # ---------------- Phase A: transposes + MADE trunk ----------------
        with tc.tile_pool(name="pa", bufs=1) as pa, \
             tc.tile_pool(name="paw", bufs=2) as paw, \
             tc.tile_pool(name="pat", bufs=2) as pat, \
             ExitStack() as actx:

            ctx_T = [pa.tile([128, bc], MMT, tag=f"ctxT{k}", name=f"ctxT{k}") for k in range(KH)]
            x_T = pa.tile([64, bc], MMT)

            w_in_t = pa.tile([64, H], MMT)
            nc.sync.dma_start(out=w_in_t[:], in_=w_in)
            wc_in_t = [pa.tile([128, H], MMT, tag=f"wci{k}", name=f"wci{k}") for k in range(KH)]
            for k in range(KH):
                nc.sync.dma_start(out=wc_in_t[k][:],
                                  in_=wc_in[k * 128:(k + 1) * 128, :])
            b1_t = pa.tile([128, KH], F32)
            nc.sync.dma_start(out=b1_t[:],
                              in_=b1.rearrange("(m p) -> p m", p=128))
            bb1_t = pa.tile([128, NBLOCKS, KH], F32)
            bb2_t = pa.tile([128, NBLOCKS, KH], F32)
            bcb_t = pa.tile([128, NBLOCKS, KH], F32)
            for tt_, src in ((bb1_t, bb1), (bb2_t, bb2), (bcb_t, bcb)):
                nc.sync.dma_start(out=tt_[:],
                                  in_=src.rearrange("i (m p) -> p i m", p=128))

            # transpose ctx and pred chunks on the PE
            pst_cm = tc.tile_pool(name="pst", bufs=2, space="PSUM")
            pst = pst_cm.__enter__()
            for c in range(nch):
                ld = pat.tile([128, C], F32, tag="ctxld", name="ctxld")
                nc.sync.dma_start(out=ld[:], in_=ctxm[c * 128:(c + 1) * 128, :])
                for k in range(KH):
                    ps = pst.tile([128, 128], F32, tag="tp", name="tp")
                    nc.tensor.transpose(ps[:], ld[:, k * 128:(k + 1) * 128],
                                        ident_t[:])
                    nc.scalar.activation(out=ctx_T[k][:, c * 128:(c + 1) * 128],
                                         in_=ps[:], func=ACTF.Copy)
                pld = pat.tile([128, F], F32, tag="predld", name="predld")
                nc.sync.dma_start(out=pld[:], in_=pred[c * 128:(c + 1) * 128, :])
                ps = pst.tile([64, 128], F32, tag="tpp", name="tpp")
                nc.tensor.transpose(ps[:], pld[:], ident_t[:])
                nc.scalar.activation(out=x_T[:, c * 128:(c + 1) * 128],
                                     in_=ps[:], func=ACTF.Copy)

            pst_cm.__exit__(None, None, None)
            psa = actx.enter_context(tc.tile_pool(name="psa", bufs=2, space="PSUM"))
            bsw = min(512, bc)
            nbs = bc // bsw
            # input projection: t = x @ (W_in*M) + ctx @ Wc_in + b1
            for bs in range(nbs):
                bsl = slice(bs * bsw, (bs + 1) * bsw)
                for m in range(KH):
                    msl = slice(m * 128, (m + 1) * 128)
                    ps = psa.tile([128, bsw], F32, tag="mm", name="mm")
                    nc.tensor.matmul(ps[:], w_in_t[:, msl],
                                     x_T[:, bsl],
                                     start=True, stop=False)
                    for k in range(KH):
                        nc.tensor.matmul(ps[:], wc_in_t[k][:, msl],
                                         ctx_T[k][:, bsl],
                                         start=False, stop=(k == KH - 1))
                    nc.scalar.activation(out=t_t[m][:, bsl], in_=ps[:],
                                         func=ACTF.Identity, bias=b1_t[:, m:m + 1])

            # residual blocks
            for i in range(NBLOCKS):
                wb1_i = [paw.tile([128, H], MMT, tag=f"wb1_{k}", name=f"wb1_{k}") for k in range(KH)]
                wb2_i = [paw.tile([128, H], MMT, tag=f"wb2_{k}", name=f"wb2_{k}") for k in range(KH)]
                wcb_i = [paw.tile([128, H], MMT, tag=f"wcb_{k}", name=f"wcb_{k}") for k in range(KH)]
                for k in range(KH):
                    ksl = slice(k * 128, (k + 1) * 128)
                    nc.sync.dma_start(out=wb1_i[k][:], in_=wb1[i, ksl, :])
                    nc.sync.dma_start(out=wb2_i[k][:], in_=wb2[i, ksl, :])
                    nc.sync.dma_start(out=wcb_i[k][:], in_=wcb[i, ksl, :])
                for bs in range(nbs):
                    bsl = slice(bs * bsw, (bs + 1) * bsw)
                    h1t = pat.tile([128, KH, bsw], MMT, tag="h1t", name="h1t")
                    for k in range(KH):
                        nc.scalar.activation(out=h1t[:, k, :],
                                             in_=t_t[k][:, bsl],
                                             func=ACTF.Relu)
                    h1 = [h1t[:, k, :] for k in range(KH)]
                    h2t = pat.tile([128, KH, bsw], MMT, tag="h2t", name="h2t")
                    h2 = [h2t[:, k, :] for k in range(KH)]
                    for m in range(KH):
                        msl = slice(m * 128, (m + 1) * 128)
                        ps = psa.tile([128, bsw], F32, tag="mm1", name="mm1")
                        for k in range(KH):
                            nc.tensor.matmul(ps[:], wb1_i[k][:, msl],
                                             h1[k],
                                             start=(k == 0), stop=(k == KH - 1))
                        nc.scalar.activation(out=h2[m], in_=ps[:],
                                             func=ACTF.Relu,
                                             bias=bb1_t[:, i, m:m + 1])
                    for m in range(KH):
                        msl = slice(m * 128, (m + 1) * 128)
                        ps2 = psa.tile([128, bsw], F32, tag="mm2", name="mm2")
                        for k in range(KH):
                            nc.tensor.matmul(ps2[:], wb2_i[k][:, msl],
                                             h2[k],
                                             start=(k == 0), stop=(k == KH - 1))
                        ps3 = psa.tile([128, bsw], F32, tag="mm3", name="mm3")
                        for k in range(KH):
                            nc.tensor.matmul(ps3[:], wcb_i[k][:, msl],
                                             ctx_T[k][:, bsl],
                                             start=(k == 0), stop=(k == KH - 1))
                        g = pat.tile([128, bsw], F32, tag="g", name="g")
                        nc.scalar.activation(out=g[:], in_=ps3[:], func=ACTF.Sigmoid,
                                             bias=bcb_t[:, i, m:m + 1])
                        v = pat.tile([128, bsw], F32, tag="v", name="v")
                        nc.vector.scalar_tensor_tensor(
                            out=v[:], in0=ps2[:], scalar=bb2_t[:, i, m:m + 1],
                            in1=g[:], op0=OP.add, op1=OP.mult)
                        nc.gpsimd.tensor_tensor(out=t_t[m][:, bsl],
                                                 in0=t_t[m][:, bsl], in1=v[:],
                                                 op=OP.add)

        # ---------------- Phase B: output GEMM + spline ----------------
        GRP = min(8, nch)
        assert nch % GRP == 0
        with tc.tile_pool(name="pb", bufs=1) as pb, \
             tc.tile_pool(name="spl", bufs=2) as spl, \
             tc.tile_pool(name="grp", bufs=2) as grp, \
             tc.tile_pool(name="psb", bufs=3, space="PSUM") as psb:

            TS = nc.vector.tensor_scalar
            TT = nc.vector.tensor_tensor
            STT = nc.vector.scalar_tensor_tensor

            def tscopy(dst, srcap):
                TS(out=dst, in0=srcap, scalar1=0.0, scalar2=None, op0=OP.add)

            for half in range(2):
                wo_t = [pb.tile([128, WOH], MMT, tag=f"wo{k}", name=f"wo{k}")
                        for k in range(KH)]
                for k in range(KH):
                    nc.sync.dma_start(
                        out=wo_t[k][:],
                        in_=w_out[k * 128:(k + 1) * 128,
                                  half * WOH:(half + 1) * WOH])
                for gidx in range(nch // GRP):
                    def gt(nm):
                        return grp.tile([128, GRP, FH], F32, tag=nm, name=nm)
                    gX = gt("gX")
                    gEWn0 = gt("gEWn0"); gEHn0 = gt("gEHn0")
                    gD0 = gt("gD0"); gD1 = gt("gD1")
                    gRall = grp.tile([128, GRP, 6, FH], F32, tag="gRall",
                                     name="gRall")

                    for gi in range(GRP):
                        c = gidx * GRP + gi
                        csl = slice(c * 128, (c + 1) * 128)
                        nc.sync.dma_start(
                            out=gX[:, gi, :],
                            in_=pred[csl, half * FH:(half + 1) * FH])
                        EW = spl.tile([128, FH, NB], F32, tag="EW", name="EW")
                        EH = spl.tile([128, FH, NB], F32, tag="EH", name="EH")
                        ED = spl.tile([128, FH, NB + 1], F32, tag="ED", name="ED")
                        for n in range(2):
                            ps = psb.tile([128, 4, 512], F32, tag="pp",
                                          name="pp", bufs=2)
                            for j in range(4):
                                nsl = slice((n * 4 + j) * 364,
                                            (n * 4 + j + 1) * 364)
                                for k in range(KH):
                                    nc.tensor.matmul(
                                        ps[:, j, 0:364],
                                        t_t[k][:, csl],
                                        wo_t[k][:, nsl],
                                        start=(k == 0), stop=(k == KH - 1))
                            psv = bass.AP(tensor=ps[:].tensor,
                                          offset=ps[:].offset,
                                          ap=[ps[:].ap[0], [512, 4], [MULT, 4],
                                              [1, MULT]])
                            fsl = slice(n * 16, (n + 1) * 16)
                            nc.scalar.activation(
                                out=EW[:, fsl, :].rearrange(
                                    "p (a f) n -> p a f n", a=4),
                                in_=psv[:, :, :, 0:NB],
                                func=ACTF.Exp, scale=SCALE)
                            nc.scalar.activation(
                                out=EH[:, fsl, :].rearrange(
                                    "p (a f) n -> p a f n", a=4),
                                in_=psv[:, :, :, NB:2 * NB],
                                func=ACTF.Exp, scale=SCALE)
                            nc.scalar.activation(
                                out=ED[:, fsl, :].rearrange(
                                    "p (a f) n -> p a f n", a=4),
                                in_=psv[:, :, :, 2 * NB:MULT],
                                func=ACTF.Exp)
                        # D = softplus(ud) = ln(exp(ud) + 1), in place over ED
                        D = ED
                        nc.scalar.activation(
                            out=D[:].rearrange("p f n -> p (f n)"),
                            in_=ED[:].rearrange("p f n -> p (f n)"),
                            func=ACTF.Ln, bias=one_t[:])
                        # per-feature sums (free-axis reduce is DVE-only)
                        Sw = spl.tile([128, FH], F32, tag="Sw", name="Sw")
                        nc.vector.tensor_reduce(out=Sw[:], in_=EW[:],
                                                axis=AX.X, op=OP.add)
                        Sh = spl.tile([128, FH], F32, tag="Sh", name="Sh")
                        nc.vector.tensor_reduce(out=Sh[:], in_=EH[:],
                                                axis=AX.X, op=OP.add)
                        CRb = spl.tile([128, FH], F32, tag="CRb", name="CRb",
                                       bufs=1)
                        nc.vector.reciprocal(out=CRb[:], in_=Sw[:])
                        TS(out=CRb[:], in0=CRb[:], scalar1=CFREE, scalar2=None,
                           op0=OP.mult)
                        CHb = spl.tile([128, FH], F32, tag="CHb", name="CHb",
                                       bufs=1)
                        nc.vector.reciprocal(out=CHb[:], in_=Sh[:])
                        TS(out=CHb[:], in0=CHb[:], scalar1=CFREE, scalar2=None,
                           op0=OP.mult)
                        # normalized widths/heights
                        EWn = spl.tile([128, FH, NB], F32, tag="EWn",
                                       name="EWn", bufs=1)
                        TT(out=EWn[:], in0=EW[:], in1=bcast(CRb[:], NB),
                           op=OP.mult)
                        EHn = spl.tile([128, FH, NB], F32, tag="EHn",
                                       name="EHn", bufs=1)
                        TT(out=EHn[:], in0=EH[:], in1=bcast(CHb[:], NB),
                           op=OP.mult)
                        dd = spl.tile([128, FH, NB], F32, tag="dd", name="dd",
                                      bufs=1)
                        TT(out=dd[:], in0=D[:, :, 1:NB + 1], in1=D[:, :, 0:NB],
                           op=OP.subtract)
                        # bin-search mask in one fused scan-compare
                        xpf = spl.tile([128, FH], F32, tag="xpf", name="xpf",
                                       bufs=1)
                        TT(out=xpf[:], in0=gX[:, gi, :], in1=fc_t[:], op=OP.add)
                        u = spl.tile([128, FH, NB], F32, tag="u", name="u",
                                     bufs=1)
                        nc.vector._custom_dve(scancmp, out=u[:], in0=EWn[:],
                                              in1=bcast(xpf[:], NB),
                                              s0=MIN_BIN)
                        # six fused masked-MAC gathers (chained; diff at ends)
                        Rbig = spl.tile([128, 6, FH, NB - 1], F32,
                                        tag="Rbig", name="Rbig", bufs=1)
                        u0 = u[:, :, 0:NB - 1]
                        streams = ((EWn[:, :, 0:NB - 1], MIN_BIN),
                                   (EWn[:, :, 1:NB], MIN_BIN),
                                   (EHn[:, :, 0:NB - 1], MIN_BIN),
                                   (EHn[:, :, 1:NB], MIN_BIN),
                                   (dd[:, :, 0:NB - 1], 0.0),
                                   (dd[:, :, 1:NB], 0.0))
                        for i_s, (t_in1, imm) in enumerate(streams):
                            nc.vector._custom_dve(scan_mac,
                                                  out=Rbig[:, i_s, :, :],
                                                  in0=u0, in1=t_in1, s0=imm)
                        # one extraction for all six gathers
                        Rl6 = bass.AP(tensor=Rbig[:].tensor,
                                      offset=Rbig[:].offset + NB - 2,
                                      ap=[Rbig[:].ap[0], [FH * (NB - 1), 6],
                                          [NB - 1, FH]])
                        tscopy(gRall[:, gi, :, :], Rl6)
                        # first-element extracts (ACT, strided)
                        nc.scalar.activation(
                            out=gEWn0[:, gi, :],
                            in_=bass.AP(tensor=EWn[:].tensor,
                                        offset=EWn[:].offset,
                                        ap=[EWn[:].ap[0], [NB, FH]]),
                            func=ACTF.Copy)
                        nc.scalar.activation(
                            out=gEHn0[:, gi, :],
                            in_=bass.AP(tensor=EHn[:].tensor,
                                        offset=EHn[:].offset,
                                        ap=[EHn[:].ap[0], [NB, FH]]),
                            func=ACTF.Copy)
                        nc.scalar.activation(
                            out=gD0[:, gi, :],
                            in_=bass.AP(tensor=D[:].tensor, offset=D[:].offset,
                                        ap=[D[:].ap[0], [NB + 1, FH]]),
                            func=ACTF.Copy)
                        nc.scalar.activation(
                            out=gD1[:, gi, :],
                            in_=bass.AP(tensor=D[:].tensor,
                                        offset=D[:].offset + 1,
                                        ap=[D[:].ap[0], [NB + 1, FH]]),
                            func=ACTF.Copy)

                    # ---- grouped small chain on [128, GRP, FH] tiles ----
                    def g2t(nm):
                        return grp.tile([128, GRP, FH], F32, tag=nm, name=nm,
                                        bufs=1)
                    # chained-scan boundary diffs: per-feature gathered values
                    gdall = grp.tile([128, GRP, 6, FH], F32, tag="gdall",
                                     name="gdall", bufs=1)
                    TT(out=gdall[:, :, :, 1:FH],
                       in0=gRall[:, :, :, 1:FH],
                       in1=gRall[:, :, :, 0:FH - 1], op=OP.subtract)
                    tscopy(gdall[:, :, :, 0:1], gRall[:, :, :, 0:1])
                    s1d = gdall[:, :, 0, :]   # in_cw (normalized left edge)
                    s2d = gdall[:, :, 1, :]
                    s3d = gdall[:, :, 2, :]   # in_ch
                    s4d = gdall[:, :, 3, :]
                    s5d = gdall[:, :, 4, :]   # D_idx - D_0
                    s6d = gdall[:, :, 5, :]   # D_{idx+1} - D_1
                    # in_w = s2d - s1d + EWn_0 + MIN_BIN
                    inw = g2t("inw")
                    TT(out=inw[:], in0=s2d, in1=s1d, op=OP.subtract)
                    STT(out=inw[:], in0=inw[:], scalar=MIN_BIN, in1=gEWn0[:],
                        op0=OP.add, op1=OP.add)
                    rw = g2t("rw")
                    nc.vector.reciprocal(out=rw[:], in_=inw[:])
                    th = g2t("th")
                    TT(out=th[:], in0=gX[:], in1=s1d, op=OP.subtract)
                    TT(out=th[:], in0=th[:], in1=rw[:], op=OP.mult)
                    inh = g2t("inh")
                    TT(out=inh[:], in0=s4d, in1=s3d, op=OP.subtract)
                    STT(out=inh[:], in0=inh[:], scalar=MIN_BIN, in1=gEHn0[:],
                        op0=OP.add, op1=OP.add)
                    dl = g2t("dl")
                    TT(out=dl[:], in0=inh[:], in1=rw[:], op=OP.mult)
                    ind = g2t("ind")
                    STT(out=ind[:], in0=s5d, scalar=MIN_DERIV, in1=gD0[:],
                        op0=OP.add, op1=OP.add)
                    indp = g2t("indp")
                    STT(out=indp[:], in0=s6d, scalar=MIN_DERIV, in1=gD1[:],
                        op0=OP.add, op1=OP.add)
                    om = g2t("om")
                    nc.scalar.activation(out=om[:], in_=th[:],
                                         func=ACTF.Identity, bias=one_t[:],
                                         scale=-1.0)
                    ttv = g2t("ttv")
                    TT(out=ttv[:], in0=th[:], in1=om[:], op=OP.mult)
                    th2 = g2t("th2")
                    nc.scalar.activation(out=th2[:], in_=th[:],
                                         func=ACTF.Square)
                    na = g2t("na")
                    TT(out=na[:], in0=dl[:], in1=th2[:], op=OP.mult)
                    nb_ = g2t("nb_")
                    TT(out=nb_[:], in0=ind[:], in1=ttv[:], op=OP.mult)
                    TT(out=na[:], in0=na[:], in1=nb_[:], op=OP.add)
                    TT(out=na[:], in0=na[:], in1=inh[:], op=OP.mult)
                    s1_ = g2t("s1_")
                    TT(out=s1_[:], in0=ind[:], in1=indp[:], op=OP.add)
                    STT(out=s1_[:], in0=dl[:], scalar=-2.0, in1=s1_[:],
                        op0=OP.mult, op1=OP.add)
                    TT(out=s1_[:], in0=s1_[:], in1=ttv[:], op=OP.mult)
                    TT(out=s1_[:], in0=s1_[:], in1=dl[:], op=OP.add)
                    rden = g2t("rden")
                    nc.vector.reciprocal(out=rden[:], in_=s1_[:])
                    cdf = g2t("cdf")
                    TT(out=cdf[:], in0=na[:], in1=rden[:], op=OP.mult)
                    TT(out=cdf[:], in0=cdf[:], in1=s3d, op=OP.add)
                    # product over the 32 features of this half
                    TT(out=cdf[:, :, 0:16], in0=cdf[:, :, 0:16],
                       in1=cdf[:, :, 16:32], op=OP.mult)
                    TT(out=cdf[:, :, 0:8], in0=cdf[:, :, 0:8],
                       in1=cdf[:, :, 8:16], op=OP.mult)
                    TT(out=cdf[:, :, 0:4], in0=cdf[:, :, 0:4],
                       in1=cdf[:, :, 4:8], op=OP.mult)
                    TT(out=cdf[:, :, 0:2], in0=cdf[:, :, 0:2],
                       in1=cdf[:, :, 2:4], op=OP.mult)
                    TT(out=halfprod[:, gidx * GRP:(gidx + 1) * GRP,
                                    half:half + 1],
                       in0=cdf[:, :, 0:1], in1=cdf[:, :, 1:2], op=OP.mult)

            fp = persist.tile([128, nch], F32)
            nc.vector.tensor_tensor(
                out=fp[:],
                in0=halfprod[:, :, 0:1].rearrange("p c h -> p (c h)"),
                in1=halfprod[:, :, 1:2].rearrange("p c h -> p (c h)"),
                op=OP.mult)
            nc.sync.dma_start(out=out_d.rearrange("(c p) -> p c", p=128),
                              in_=fp[:])

    nc.compile()
    return nc


def _prep_shared(W_in, b_in, Wc_in, bc_in, Wb1, bb1, Wb2, bb2, Wcb, bcb,
                 W_out, b_out, mm_dt):
    m_in, m_hh, m_out = _masks()
    assert not np.any(b_out), "nonzero b_out not supported by this kernel"
    rnd = _round_fp32r if mm_dt == mybir.dt.float32r else (
        lambda a: np.ascontiguousarray(a, dtype=np.float32))
    shared = {
        "w_in": rnd(W_in * m_in),
        "wc_in": rnd(Wc_in),
        "wb1": rnd(Wb1 * m_hh[None]),
        "wb2": rnd(Wb2 * m_hh[None]),
        "wcb": rnd(Wcb),
        "w_out": rnd(W_out * m_out),
        "b1": np.ascontiguousarray((b_in + bc_in).astype(np.float32)),
        "bb1": np.ascontiguousarray(bb1.astype(np.float32)),
        "bb2": np.ascontiguousarray(bb2.astype(np.float32)),
        "bcb": np.ascontiguousarray(bcb.astype(np.float32)),
        "ident": np.eye(128, dtype=np.float32),
        "fcon": np.arange(FH, dtype=np.float32),
    }
    return shared


def kernel(predicates, contexts, W_in, b_in, Wc_in, bc_in, Wb1, bb1, Wb2, bb2,
           Wcb, bcb, W_out, b_out):
    global LAST_RESULTS
    predicates = np.asarray(predicates, dtype=np.float32)
    contexts = np.asarray(contexts, dtype=np.float32)
    bc = predicates.shape[0] // NCORES
    key = (bc, MM_DT)
    if key not in _CACHE:
        _CACHE[key] = _build(bc, MM_DT)
    nc = _CACHE[key]
    shared = _prep_shared(W_in, b_in, Wc_in, bc_in, Wb1, bb1, Wb2, bb2,
                          Wcb, bcb, W_out, b_out, MM_DT)
    in_maps = []
    for cid in range(NCORES):
        sl = slice(cid * bc, (cid + 1) * bc)
        m = dict(shared)
        m["pred"] = np.ascontiguousarray(predicates[sl])
        m["ctx"] = np.ascontiguousarray(contexts[sl])
        in_maps.append(m)
    res = run_bass_kernel_spmd(nc, in_maps, core_ids=list(range(NCORES)),
                               trace=TRACE)
    LAST_RESULTS = res
    return np.concatenate([res.results[i]["out"] for i in range(NCORES)])


# revision 9
# speedup vs baseline: 1.0636x; 1.0001x over previous
"""Trainium2 Bass kernel for nn_AutoregressiveCDF (MADE + rational-quadratic
spline CDF, product over features).

Strategy: pure data-parallel over 8 NeuronCores (batch 16384 -> 8 x 2048),
weights replicated.  Per core:
  Phase A: transpose predicates/contexts via PE, run the MADE trunk as
           hidden-on-partition GEMMs (fp32r on the PE), activations on ACT.
  Phase B: output GEMM t @ W_out in [batch-part, feature-free] orientation,
           PSUM consumed directly by ACT exp; spline evaluated in the
           *normalized* domain: widths scaled by CFREE/Sw so each feature's
           edge span is exactly 1.0, making the chained running edge value
           at feature f equal f + local_edge.  A fused scan-compare custom
           DVE op then yields the bin mask u in one pass (compare against
           x + f), and six fused scan-MAC ops produce the gathered spline
           parameters (prefix sums at the bin index via segment-boundary
           diffs).  The min-bin affine is folded into the scans (imm2), so
           no bin-index tensor, searchsorted gather, or edge tensor is ever
           materialized.
"""

import numpy as np
from contextlib import ExitStack

import concourse.bass as bass
import concourse.bacc as bacc
import concourse.tile as tile
from concourse import mybir
from concourse.bass_utils import run_bass_kernel_spmd

F32 = mybir.dt.float32

# problem sizes (hardcoded per contract)
B, F, H, C = 16384, 64, 512, 512
NB = 30
MULT = 3 * NB + 1            # 91
NBLOCKS = 3
NCORES = 8
MIN_BIN = 1e-3
MIN_DERIV = 1e-3
CFREE = float(1.0 - MIN_BIN * NB)         # softmax mass after min-bin affine
SCALE = float(np.float32(1.0 / np.sqrt(H)))
FH = F // 2                  # features per half (32)
WOH = FH * MULT              # 2912 W_out cols per half
KH = H // 128                # 4 hidden chunks

# knobs (test.py may override module globals before calling kernel())
MM_DT = mybir.dt.float32r    # PE dtype: float32r (fast) or float32 (safe)
TRACE = False
LAST_RESULTS = None          # BassKernelResults of the most recent run

_CACHE = {}


def _masks():
    d_in = np.arange(1, F + 1)
    d_h = np.arange(H) % max(1, F - 1) + min(1, F - 1)
    m_in = (d_h[None, :] >= d_in[:, None]).astype(np.float32)
    m_hh = (d_h[None, :] >= d_h[:, None]).astype(np.float32)
    d_out = np.repeat(d_in, MULT)
    m_out = (d_out[None, :] > d_h[:, None]).astype(np.float32)
    return m_in, m_hh, m_out


def _scanmac_ref(in0, in1, s0, s1, imm2):
    a = np.asarray(in0, np.float32).reshape(np.asarray(in0).shape[0], -1)
    b = np.asarray(in1, np.float32).reshape(a.shape)
    return np.cumsum(a * (b + np.float32(s0)), axis=1,
                     dtype=np.float32).reshape(np.asarray(in0).shape)


def _scancmp_ref(in0, in1, s0, s1, imm2):
    a = np.asarray(in0, np.float32).reshape(np.asarray(in0).shape[0], -1)
    t = np.asarray(in1, np.float32).reshape(a.shape)
    s = np.cumsum(a + np.float32(s0), axis=1, dtype=np.float32)
    return (t >= s).astype(np.float32).reshape(np.asarray(in0).shape)


def _register_spline_ops():
    """Register the two fused DVE ops the spline needs:

    SCAN_MAC_ANT: out = cumsum(in0 * (in1 + s0))   (chained masked MAC)
    SCANCMP_ANT:  out = (in1 >= cumsum(in0 + s0))  (bin-search mask)
    """
    import concourse.dve_ops as dve_ops
    from concourse.dve_spec import Spec, Src0, Src1, C0, scan, AluOp, lower
    from concourse.dve_uop import DveOpSpec
    have = {op.name: op for op in dve_ops.OPS}
    if "SCAN_MAC_ANT" in have and "SCANCMP_ANT" in have:
        return have["SCAN_MAC_ANT"], have["SCANCMP_ANT"]

    def reg(name, spec):
        row = max(dve_ops._SUB_OPCODE_FOR_NAME.values()) + 1
        assert row < 0x20
        shas = {}
        for ver in ("v3", "v4"):
            u = lower(spec, ver=ver)
            shas[ver] = DveOpSpec(name=name, opcode=row, uops=u,
                                  rd1_en=True).sha(ver)
        op = dve_ops.DveOp(name, spec, subdim=False, uops_sha=shas)
        dve_ops.OPS.append(op)
        dve_ops.CUSTOM_DVE_SPECS[name] = spec
        dve_ops._SUB_OPCODE_FOR_NAME[name] = row
        return op

    mac = reg("SCAN_MAC_ANT",
              Spec(body=scan(AluOp.ADD, Src0 * (Src1 + C0)),
                   reference=_scanmac_ref))
    cmp_ = reg("SCANCMP_ANT",
               Spec(body=scan(AluOp.ADD, Src0 + C0) <= Src1,
                    reference=_scancmp_ref))
    return mac, cmp_


class _Bacc(bacc.Bacc):
    """Bacc with a trimmed activation-table list so Exp and Ln share one
    table (no per-chunk ACT_TABLE_LOAD thrash)."""

    _KEEP_TABLES = ("natural_log_exp_and_others", "sigmoid_and_others")

    def insert_act_table_loads(self):
        import bass_rust as _bass_rust
        from concourse.hw_specs import get_activation_tables
        import concourse.mybir as _mb
        has_activation = any(
            isinstance(i, _mb.InstActivation)
            for b in self.main_func.blocks
            for i in b.instructions
        )
        if not has_activation:
            return
        # act_func_set_id is positional in act_info.json order: keep every
        # entry but empty the unwanted ones so the chooser can't pick them.
        all_tables = get_activation_tables(self.m.arch)
        tables = [(k, (v if k in self._KEEP_TABLES else set()))
                  for k, v in all_tables.items()]
        _bass_rust.insert_act_table_loads(self, tables)


def _round_fp32r(a):
    """Round fp32 to the PE's fp32r grid (1s+8e+11m, RNE) on the host."""
    b = np.ascontiguousarray(a, dtype=np.float32).view(np.uint32)
    lsb = (b >> 12) & np.uint32(1)
    b2 = ((b + np.uint32(0x7FF) + lsb) & np.uint32(0xFFFFF000)).astype(np.uint32)
    return b2.view(np.float32)


def _build(bc, mm_dt):
    """Build the per-core Bass module for bc batch rows per core."""
    nch = bc // 128
    MMT = mm_dt
    scan_mac, scancmp = _register_spline_ops()
    nc = _Bacc("TRN2", target_bir_lowering=False, debug=False,
               enable_asserts=False)

    def din(name, shape, dt=F32):
        return nc.dram_tensor(name, list(shape), dt, kind="ExternalInput").ap()

    pred = din("pred", (bc, F))
    ctxm = din("ctx", (bc, C))
    w_in = din("w_in", (F, H), MMT)
    wc_in = din("wc_in", (C, H), MMT)
    wb1 = din("wb1", (NBLOCKS, H, H), MMT)
    wb2 = din("wb2", (NBLOCKS, H, H), MMT)
    wcb = din("wcb", (NBLOCKS, C, H), MMT)
    w_out = din("w_out", (H, F * MULT), MMT)
    b1 = din("b1", (H,))
    bb1 = din("bb1", (NBLOCKS, H))
    bb2 = din("bb2", (NBLOCKS, H))
    bcb = din("bcb", (NBLOCKS, H))
    ident = din("ident", (128, 128))
    fcon = din("fcon", (FH,))
    out_d = nc.dram_tensor("out", [bc], F32, kind="ExternalOutput").ap()

    AX = mybir.AxisListType
    OP = mybir.AluOpType
    ACTF = mybir.ActivationFunctionType

    def bcast(ap2d, n):
        """[P, M] AP -> [P, M, n] with stride-0 inner (broadcast along bins)."""
        return bass.AP(tensor=ap2d.tensor, offset=ap2d.offset,
                       ap=list(ap2d.ap) + [[0, n]])

    def pbcast(ap1d, p, n):
        """[n] DRAM AP -> [p, n] with stride-0 partitions (for DMA)."""
        return bass.AP(tensor=ap1d.tensor, offset=ap1d.offset,
                       ap=[[0, p]] + list(ap1d.ap))

    with tile.TileContext(nc) as tc, ExitStack() as ctx:
        const = ctx.enter_context(tc.tile_pool(name="const", bufs=1))
        persist = ctx.enter_context(tc.tile_pool(name="persist", bufs=1))

        ident_t = const.tile([128, 128], F32)
        nc.sync.dma_start(out=ident_t[:], in_=ident)
        fc_t = const.tile([128, FH], F32)
        nc.sync.dma_start(out=fc_t[:], in_=pbcast(fcon, 128, FH))
        one_t = const.tile([128, 1], F32)
        nc.vector.memset(one_t[:], 1.0)

        # persistent activations
        t_t = [persist.tile([128, bc], MMT, tag=f"t{k}", name=f"t{k}") for k in range(KH)]
        halfprod = persist.tile([128, nch, 2], F32)

        # ---------------- Phase A: transposes + MADE trunk ----------------
        with tc.tile_pool(name="pa", bufs=1) as pa, \
             tc.tile_pool(name="paw", bufs=2) as paw, \
             tc.tile_pool(name="pat", bufs=2) as pat, \
             ExitStack() as actx:

            ctx_T = [pa.tile([128, bc], MMT, tag=f"ctxT{k}", name=f"ctxT{k}") for k in range(KH)]
            x_T = pa.tile([64, bc], MMT)

            w_in_t = pa.tile([64, H], MMT)
            nc.sync.dma_start(out=w_in_t[:], in_=w_in)
            wc_in_t = [pa.tile([128, H], MMT, tag=f"wci{k}", name=f"wci{k}") for k in range(KH)]
            for k in range(KH):
                nc.sync.dma_start(out=wc_in_t[k][:],
                                  in_=wc_in[k * 128:(k + 1) * 128, :])
            b1_t = pa.tile([128, KH], F32)
            nc.sync.dma_start(out=b1_t[:],
                              in_=b1.rearrange("(m p) -> p m", p=128))
            bb1_t = pa.tile([128, NBLOCKS, KH], F32)
            bb2_t = pa.tile([128, NBLOCKS, KH], F32)
            bcb_t = pa.tile([128, NBLOCKS, KH], F32)
            for tt_, src in ((bb1_t, bb1), (bb2_t, bb2), (bcb_t, bcb)):
                nc.sync.dma_start(out=tt_[:],
                                  in_=src.rearrange("i (m p) -> p i m", p=128))

            # transpose ctx and pred chunks on the PE
            pst_cm = tc.tile_pool(name="pst", bufs=2, space="PSUM")
            pst = pst_cm.__enter__()
            for c in range(nch):
                ld = pat.tile([128, C], F32, tag="ctxld", name="ctxld")
                nc.sync.dma_start(out=ld[:], in_=ctxm[c * 128:(c + 1) * 128, :])
                for k in range(KH):
                    ps = pst.tile([128, 128], F32, tag="tp", name="tp")
                    nc.tensor.transpose(ps[:], ld[:, k * 128:(k + 1) * 128],
                                        ident_t[:])
                    nc.scalar.activation(out=ctx_T[k][:, c * 128:(c + 1) * 128],
                                         in_=ps[:], func=ACTF.Copy)
                pld = pat.tile([128, F], F32, tag="predld", name="predld")
                nc.sync.dma_start(out=pld[:], in_=pred[c * 128:(c + 1) * 128, :])
                ps = pst.tile([64, 128], F32, tag="tpp", name="tpp")
                nc.tensor.transpose(ps[:], pld[:], ident_t[:])
                nc.scalar.activation(out=x_T[:, c * 128:(c + 1) * 128],
                                     in_=ps[:], func=ACTF.Copy)

            pst_cm.__exit__(None, None, None)
            psa = actx.enter_context(tc.tile_pool(name="psa", bufs=2, space="PSUM"))
            bsw = min(512, bc)
            nbs = bc // bsw
            # input projection: t = x @ (W_in*M) + ctx @ Wc_in + b1
            for bs in range(nbs):
                bsl = slice(bs * bsw, (bs + 1) * bsw)
                for m in range(KH):
                    msl = slice(m * 128, (m + 1) * 128)
                    ps = psa.tile([128, bsw], F32, tag="mm", name="mm")
                    nc.tensor.matmul(ps[:], w_in_t[:, msl],
                                     x_T[:, bsl],
                                     start=True, stop=False)
                    for k in range(KH):
                        nc.tensor.matmul(ps[:], wc_in_t[k][:, msl],
                                         ctx_T[k][:, bsl],
                                         start=False, stop=(k == KH - 1))
                    nc.scalar.activation(out=t_t[m][:, bsl], in_=ps[:],
                                         func=ACTF.Identity, bias=b1_t[:, m:m + 1])

            # residual blocks
            for i in range(NBLOCKS):
                wb1_i = [paw.tile([128, H], MMT, tag=f"wb1_{k}", name=f"wb1_{k}") for k in range(KH)]
                wb2_i = [paw.tile([128, H], MMT, tag=f"wb2_{k}", name=f"wb2_{k}") for k in range(KH)]
                wcb_i = [paw.tile([128, H], MMT, tag=f"wcb_{k}", name=f"wcb_{k}") for k in range(KH)]
                for k in range(KH):
                    ksl = slice(k * 128, (k + 1) * 128)
                    nc.sync.dma_start(out=wb1_i[k][:], in_=wb1[i, ksl, :])
                    nc.sync.dma_start(out=wb2_i[k][:], in_=wb2[i, ksl, :])
                    nc.sync.dma_start(out=wcb_i[k][:], in_=wcb[i, ksl, :])
                for bs in range(nbs):
                    bsl = slice(bs * bsw, (bs + 1) * bsw)
                    h1t = pat.tile([128, KH, bsw], MMT, tag="h1t", name="h1t")
                    for k in range(KH):
                        nc.scalar.activation(out=h1t[:, k, :],
                                             in_=t_t[k][:, bsl],
                                             func=ACTF.Relu)
                    h1 = [h1t[:, k, :] for k in range(KH)]
                    h2t = pat.tile([128, KH, bsw], MMT, tag="h2t", name="h2t")
                    h2 = [h2t[:, k, :] for k in range(KH)]
                    for m in range(KH):
                        msl = slice(m * 128, (m + 1) * 128)
                        ps = psa.tile([128, bsw], F32, tag="mm1", name="mm1")
                        for k in range(KH):
                            nc.tensor.matmul(ps[:], wb1_i[k][:, msl],
                                             h1[k],
                                             start=(k == 0), stop=(k == KH - 1))
                        nc.scalar.activation(out=h2[m], in_=ps[:],
                                             func=ACTF.Relu,
                                             bias=bb1_t[:, i, m:m + 1])
                    for m in range(KH):
                        msl = slice(m * 128, (m + 1) * 128)
                        ps2 = psa.tile([128, bsw], F32, tag="mm2", name="mm2")
                        for k in range(KH):
                            nc.tensor.matmul(ps2[:], wb2_i[k][:, msl],
                                             h2[k],
                                             start=(k == 0), stop=(k == KH - 1))
                        ps3 = psa.tile([128, bsw], F32, tag="mm3", name="mm3")
                        for k in range(KH):
                            nc.tensor.matmul(ps3[:], wcb_i[k][:, msl],
                                             ctx_T[k][:, bsl],
                                             start=(k == 0), stop=(k == KH - 1))
                        g = pat.tile([128, bsw], F32, tag="g", name="g")
                        nc.scalar.activation(out=g[:], in_=ps3[:], func=ACTF.Sigmoid,
                                             bias=bcb_t[:, i, m:m + 1])
                        v = pat.tile([128, bsw], F32, tag="v", name="v")
                        nc.vector.scalar_tensor_tensor(
                            out=v[:], in0=ps2[:], scalar=bb2_t[:, i, m:m + 1],
                            in1=g[:], op0=OP.add, op1=OP.mult)
                        nc.gpsimd.tensor_tensor(out=t_t[m][:, bsl],
                                                 in0=t_t[m][:, bsl], in1=v[:],
                                                 op=OP.add)

        # ---------------- Phase B: output GEMM + spline ----------------
        GRP = min(8, nch)
        assert nch % GRP == 0
        with tc.tile_pool(name="pb", bufs=1) as pb, \
             tc.tile_pool(name="spl", bufs=2) as spl, \
             tc.tile_pool(name="grp", bufs=2) as grp, \
             tc.tile_pool(name="psb", bufs=3, space="PSUM") as psb:

            TS = nc.vector.tensor_scalar
            TT = nc.vector.tensor_tensor
            STT = nc.vector.scalar_tensor_tensor

            def tscopy(dst, srcap):
                TS(out=dst, in0=srcap, scalar1=0.0, scalar2=None, op0=OP.add)

            for half in range(2):
                wo_t = [pb.tile([128, WOH], MMT, tag=f"wo{k}", name=f"wo{k}")
                        for k in range(KH)]
                for k in range(KH):
                    nc.sync.dma_start(
                        out=wo_t[k][:],
                        in_=w_out[k * 128:(k + 1) * 128,
                                  half * WOH:(half + 1) * WOH])
                for gidx in range(nch // GRP):
                    def gt(nm):
                        return grp.tile([128, GRP, FH], F32, tag=nm, name=nm)
                    gX = gt("gX")
                    gEWn0 = gt("gEWn0"); gEHn0 = gt("gEHn0")
                    gD0 = gt("gD0"); gD1 = gt("gD1")
                    gRall = grp.tile([128, GRP, 6, FH], F32, tag="gRall",
                                     name="gRall")

                    for gi in range(GRP):
                        c = gidx * GRP + gi
                        csl = slice(c * 128, (c + 1) * 128)
                        nc.sync.dma_start(
                            out=gX[:, gi, :],
                            in_=pred[csl, half * FH:(half + 1) * FH])
                        EW = spl.tile([128, FH, NB], F32, tag="EW", name="EW")
                        EH = spl.tile([128, FH, NB], F32, tag="EH", name="EH")
                        ED = spl.tile([128, FH, NB + 1], F32, tag="ED", name="ED")
                        for n in range(2):
                            ps = psb.tile([128, 4, 512], F32, tag="pp",
                                          name="pp", bufs=2)
                            for j in range(4):
                                nsl = slice((n * 4 + j) * 364,
                                            (n * 4 + j + 1) * 364)
                                for k in range(KH):
                                    nc.tensor.matmul(
                                        ps[:, j, 0:364],
                                        t_t[k][:, csl],
                                        wo_t[k][:, nsl],
                                        start=(k == 0), stop=(k == KH - 1))
                            psv = bass.AP(tensor=ps[:].tensor,
                                          offset=ps[:].offset,
                                          ap=[ps[:].ap[0], [512, 4], [MULT, 4],
                                              [1, MULT]])
                            fsl = slice(n * 16, (n + 1) * 16)
                            nc.scalar.activation(
                                out=EW[:, fsl, :].rearrange(
                                    "p (a f) n -> p a f n", a=4),
                                in_=psv[:, :, :, 0:NB],
                                func=ACTF.Exp, scale=SCALE)
                            nc.scalar.activation(
                                out=EH[:, fsl, :].rearrange(
                                    "p (a f) n -> p a f n", a=4),
                                in_=psv[:, :, :, NB:2 * NB],
                                func=ACTF.Exp, scale=SCALE)
                            nc.scalar.activation(
                                out=ED[:, fsl, :].rearrange(
                                    "p (a f) n -> p a f n", a=4),
                                in_=psv[:, :, :, 2 * NB:MULT],
                                func=ACTF.Exp)
                        # D = softplus(ud) = ln(exp(ud) + 1), in place over ED
                        D = ED
                        nc.scalar.activation(
                            out=D[:].rearrange("p f n -> p (f n)"),
                            in_=ED[:].rearrange("p f n -> p (f n)"),
                            func=ACTF.Ln, bias=one_t[:])
                        # per-feature sums (free-axis reduce is DVE-only)
                        Sw = spl.tile([128, FH], F32, tag="Sw", name="Sw")
                        nc.vector.tensor_reduce(out=Sw[:], in_=EW[:],
                                                axis=AX.X, op=OP.add)
                        Sh = spl.tile([128, FH], F32, tag="Sh", name="Sh")
                        nc.vector.tensor_reduce(out=Sh[:], in_=EH[:],
                                                axis=AX.X, op=OP.add)
                        CRb = spl.tile([128, FH], F32, tag="CRb", name="CRb",
                                       bufs=1)
                        nc.vector.reciprocal(out=CRb[:], in_=Sw[:])
                        TS(out=CRb[:], in0=CRb[:], scalar1=CFREE, scalar2=None,
                           op0=OP.mult)
                        CHb = spl.tile([128, FH], F32, tag="CHb", name="CHb",
                                       bufs=1)
                        nc.vector.reciprocal(out=CHb[:], in_=Sh[:])
                        TS(out=CHb[:], in0=CHb[:], scalar1=CFREE, scalar2=None,
                           op0=OP.mult)
                        # normalized widths/heights
                        EWn = spl.tile([128, FH, NB], F32, tag="EWn",
                                       name="EWn", bufs=1)
                        TT(out=EWn[:], in0=EW[:], in1=bcast(CRb[:], NB),
                           op=OP.mult)
                        EHn = spl.tile([128, FH, NB], F32, tag="EHn",
                                       name="EHn", bufs=1)
                        TT(out=EHn[:], in0=EH[:], in1=bcast(CHb[:], NB),
                           op=OP.mult)
                        dd = spl.tile([128, FH, NB], F32, tag="dd", name="dd",
                                      bufs=1)
                        nc.gpsimd.tensor_tensor(out=dd[:],
                                                in0=D[:, :, 1:NB + 1],
                                                in1=D[:, :, 0:NB],
                                                op=OP.subtract)
                        # bin-search mask in one fused scan-compare
                        xpf = spl.tile([128, FH], F32, tag="xpf", name="xpf",
                                       bufs=1)
                        TT(out=xpf[:], in0=gX[:, gi, :], in1=fc_t[:], op=OP.add)
                        u = spl.tile([128, FH, NB], F32, tag="u", name="u",
                                     bufs=1)
                        nc.vector._custom_dve(scancmp, out=u[:], in0=EWn[:],
                                              in1=bcast(xpf[:], NB),
                                              s0=MIN_BIN)
                        # six fused masked-MAC gathers (chained; diff at ends)
                        Rbig = spl.tile([128, 6, FH, NB - 1], F32,
                                        tag="Rbig", name="Rbig", bufs=1)
                        u0 = u[:, :, 0:NB - 1]
                        streams = ((EWn[:, :, 0:NB - 1], MIN_BIN),
                                   (EWn[:, :, 1:NB], MIN_BIN),
                                   (EHn[:, :, 0:NB - 1], MIN_BIN),
                                   (EHn[:, :, 1:NB], MIN_BIN),
                                   (dd[:, :, 0:NB - 1], 0.0),
                                   (dd[:, :, 1:NB], 0.0))
                        for i_s, (t_in1, imm) in enumerate(streams):
                            nc.vector._custom_dve(scan_mac,
                                                  out=Rbig[:, i_s, :, :],
                                                  in0=u0, in1=t_in1, s0=imm)
                        # one extraction for all six gathers
                        Rl6 = bass.AP(tensor=Rbig[:].tensor,
                                      offset=Rbig[:].offset + NB - 2,
                                      ap=[Rbig[:].ap[0], [FH * (NB - 1), 6],
                                          [NB - 1, FH]])
                        tscopy(gRall[:, gi, :, :], Rl6)
                        # first-element extracts (ACT, strided)
                        nc.scalar.activation(
                            out=gEWn0[:, gi, :],
                            in_=bass.AP(tensor=EWn[:].tensor,
                                        offset=EWn[:].offset,
                                        ap=[EWn[:].ap[0], [NB, FH]]),
                            func=ACTF.Copy)
                        nc.scalar.activation(
                            out=gEHn0[:, gi, :],
                            in_=bass.AP(tensor=EHn[:].tensor,
                                        offset=EHn[:].offset,
                                        ap=[EHn[:].ap[0], [NB, FH]]),
                            func=ACTF.Copy)
                        nc.scalar.activation(
                            out=gD0[:, gi, :],
                            in_=bass.AP(tensor=D[:].tensor, offset=D[:].offset,
                                        ap=[D[:].ap[0], [NB + 1, FH]]),
                            func=ACTF.Copy)
                        nc.scalar.activation(
                            out=gD1[:, gi, :],
                            in_=bass.AP(tensor=D[:].tensor,
                                        offset=D[:].offset + 1,
                                        ap=[D[:].ap[0], [NB + 1, FH]]),
                            func=ACTF.Copy)

                    # ---- grouped small chain on [128, GRP, FH] tiles ----
                    def g2t(nm):
                        return grp.tile([128, GRP, FH], F32, tag=nm, name=nm,
                                        bufs=1)
                    # chained-scan boundary diffs: per-feature gathered values
                    gdall = grp.tile([128, GRP, 6, FH], F32, tag="gdall",
                                     name="gdall", bufs=1)
                    TT(out=gdall[:, :, :, 1:FH],
                       in0=gRall[:, :, :, 1:FH],
                       in1=gRall[:, :, :, 0:FH - 1], op=OP.subtract)
                    tscopy(gdall[:, :, :, 0:1], gRall[:, :, :, 0:1])
                    s1d = gdall[:, :, 0, :]   # in_cw (normalized left edge)
                    s2d = gdall[:, :, 1, :]
                    s3d = gdall[:, :, 2, :]   # in_ch
                    s4d = gdall[:, :, 3, :]
                    s5d = gdall[:, :, 4, :]   # D_idx - D_0
                    s6d = gdall[:, :, 5, :]   # D_{idx+1} - D_1
                    # in_w = s2d - s1d + EWn_0 + MIN_BIN
                    inw = g2t("inw")
                    TT(out=inw[:], in0=s2d, in1=s1d, op=OP.subtract)
                    STT(out=inw[:], in0=inw[:], scalar=MIN_BIN, in1=gEWn0[:],
                        op0=OP.add, op1=OP.add)
                    rw = g2t("rw")
                    nc.vector.reciprocal(out=rw[:], in_=inw[:])
                    th = g2t("th")
                    TT(out=th[:], in0=gX[:], in1=s1d, op=OP.subtract)
                    TT(out=th[:], in0=th[:], in1=rw[:], op=OP.mult)
                    inh = g2t("inh")
                    TT(out=inh[:], in0=s4d, in1=s3d, op=OP.subtract)
                    STT(out=inh[:], in0=inh[:], scalar=MIN_BIN, in1=gEHn0[:],
                        op0=OP.add, op1=OP.add)
                    dl = g2t("dl")
                    TT(out=dl[:], in0=inh[:], in1=rw[:], op=OP.mult)
                    ind = g2t("ind")
                    STT(out=ind[:], in0=s5d, scalar=MIN_DERIV, in1=gD0[:],
                        op0=OP.add, op1=OP.add)
                    indp = g2t("indp")
                    STT(out=indp[:], in0=s6d, scalar=MIN_DERIV, in1=gD1[:],
                        op0=OP.add, op1=OP.add)
                    om = g2t("om")
                    TS(out=om[:], in0=th[:], scalar1=-1.0, scalar2=1.0,
                       op0=OP.mult, op1=OP.add)
                    ttv = g2t("ttv")
                    TT(out=ttv[:], in0=th[:], in1=om[:], op=OP.mult)
                    th2 = g2t("th2")
                    TT(out=th2[:], in0=th[:], in1=th[:], op=OP.mult)
                    na = g2t("na")
                    TT(out=na[:], in0=dl[:], in1=th2[:], op=OP.mult)
                    nb_ = g2t("nb_")
                    TT(out=nb_[:], in0=ind[:], in1=ttv[:], op=OP.mult)
                    TT(out=na[:], in0=na[:], in1=nb_[:], op=OP.add)
                    TT(out=na[:], in0=na[:], in1=inh[:], op=OP.mult)
                    s1_ = g2t("s1_")
                    TT(out=s1_[:], in0=ind[:], in1=indp[:], op=OP.add)
                    STT(out=s1_[:], in0=dl[:], scalar=-2.0, in1=s1_[:],
                        op0=OP.mult, op1=OP.add)
                    TT(out=s1_[:], in0=s1_[:], in1=ttv[:], op=OP.mult)
                    TT(out=s1_[:], in0=s1_[:], in1=dl[:], op=OP.add)
                    rden = g2t("rden")
                    nc.vector.reciprocal(out=rden[:], in_=s1_[:])
                    cdf = g2t("cdf")
                    TT(out=cdf[:], in0=na[:], in1=rden[:], op=OP.mult)
                    TT(out=cdf[:], in0=cdf[:], in1=s3d, op=OP.add)
                    # product over the 32 features of this half
                    TT(out=cdf[:, :, 0:16], in0=cdf[:, :, 0:16],
                       in1=cdf[:, :, 16:32], op=OP.mult)
                    TT(out=cdf[:, :, 0:8], in0=cdf[:, :, 0:8],
                       in1=cdf[:, :, 8:16], op=OP.mult)
                    TT(out=cdf[:, :, 0:4], in0=cdf[:, :, 0:4],
                       in1=cdf[:, :, 4:8], op=OP.mult)
                    TT(out=cdf[:, :, 0:2], in0=cdf[:, :, 0:2],
                       in1=cdf[:, :, 2:4], op=OP.mult)
                    TT(out=halfprod[:, gidx * GRP:(gidx + 1) * GRP,
                                    half:half + 1],
                       in0=cdf[:, :, 0:1], in1=cdf[:, :, 1:2], op=OP.mult)

            fp = persist.tile([128, nch], F32)
            nc.vector.tensor_tensor(
                out=fp[:],
                in0=halfprod[:, :, 0:1].rearrange("p c h -> p (c h)"),
                in1=halfprod[:, :, 1:2].rearrange("p c h -> p (c h)"),
                op=OP.mult)
            nc.sync.dma_start(out=out_d.rearrange("(c p) -> p c", p=128),
                              in_=fp[:])

    nc.compile()
    return nc


def _prep_shared(W_in, b_in, Wc_in, bc_in, Wb1, bb1, Wb2, bb2, Wcb, bcb,
                 W_out, b_out, mm_dt):
    m_in, m_hh, m_out = _masks()
    assert not np.any(b_out), "nonzero b_out not supported by this kernel"
    rnd = _round_fp32r if mm_dt == mybir.dt.float32r else (
        lambda a: np.ascontiguousarray(a, dtype=np.float32))
    shared = {
        "w_in": rnd(W_in * m_in),
        "wc_in": rnd(Wc_in),
        "wb1": rnd(Wb1 * m_hh[None]),
        "wb2": rnd(Wb2 * m_hh[None]),
        "wcb": rnd(Wcb),
        "w_out": rnd(W_out * m_out),
        "b1": np.ascontiguousarray((b_in + bc_in).astype(np.float32)),
        "bb1": np.ascontiguousarray(bb1.astype(np.float32)),
        "bb2": np.ascontiguousarray(bb2.astype(np.float32)),
        "bcb": np.ascontiguousarray(bcb.astype(np.float32)),
        "ident": np.eye(128, dtype=np.float32),
        "fcon": np.arange(FH, dtype=np.float32),
    }
    return shared


def kernel(predicates, contexts, W_in, b_in, Wc_in, bc_in, Wb1, bb1, Wb2, bb2,
           Wcb, bcb, W_out, b_out):
    global LAST_RESULTS
    predicates = np.asarray(predicates, dtype=np.float32)
    contexts = np.asarray(contexts, dtype=np.float32)
    bc = predicates.shape[0] // NCORES
    key = (bc, MM_DT)
    if key not in _CACHE:
        _CACHE[key] = _build(bc, MM_DT)
    nc = _CACHE[key]
    shared = _prep_shared(W_in, b_in, Wc_in, bc_in, Wb1, bb1, Wb2, bb2,
                          Wcb, bcb, W_out, b_out, MM_DT)
    in_maps = []
    for cid in range(NCORES):
        sl = slice(cid * bc, (cid + 1) * bc)
        m = dict(shared)
        m["pred"] = np.ascontiguousarray(predicates[sl])
        m["ctx"] = np.ascontiguousarray(contexts[sl])
        in_maps.append(m)
    res = run_bass_kernel_spmd(nc, in_maps, core_ids=list(range(NCORES)),
                               trace=TRACE)
    LAST_RESULTS = res
    return np.concatenate([res.results[i]["out"] for i in range(NCORES)])
